# revision 2
# baseline (speedup 1.0000x reference)
"""Trainium2 Bass kernel for nn_Clustering_Layer (retrieval_knn).

Computes q = row_normalize(1 / (1 + ||z - c_k||^2)) for z:[N,D], c:[K,D]
(Student-t / DEC clustering assignment, alpha=1).

Strategy (8 NeuronCores, data parallel over N):
  - Host: shard z along N; pre-transpose each shard to zT [D, N/8] (bf16) so
    the PE stationary operand loads directly from natural DRAM layout.
    Fold the distance expansion into one PSUM accumulation:
        denom = 1 + ||z||^2 + ||c_k||^2 - 2 z.c_k
    via 3 matmuls per 128-row tile:
        chunk A: zT[0:128]  x (-2 c^T)[0:128]
        chunk B: zT[128:256]x (-2 c^T)[128:256]
        aug    : [z2_hi; z2_lo; ones] x [ones; ones; 1+c2]
    z2 = ||z||^2 is computed on host in fp32 and shipped as two bf16 rows
    (hi + lo) so the dominant term keeps ~fp32 accuracy.
  - Device epilogue per group of 16 row-tiles (batched to amortize fixed
    per-instruction overheads):  ACT Ln -> ACT Exp(-x) (= 1/denom; the DVE
    reciprocal is the only alternative and is slower / this keeps DVE free),
    DVE row-sum, DVE reciprocal of sums, DVE broadcast multiply.
  - Store q rows in natural [N,100] f32 layout.
"""

import os
import sys
from contextlib import ExitStack

import numpy as np

for _p in ("/opt/trn_rl_repo", "/root/.axon_site/_ro/trn_rl_repo"):
    if os.path.isdir(_p) and _p not in sys.path:
        sys.path.insert(0, _p)

import ml_dtypes  # noqa: E402

import concourse.bass as bass  # noqa: E402
import concourse.tile as tile  # noqa: E402
from concourse import bacc, bass_utils, mybir  # noqa: E402

# Problem shape (hardcoded per spec).
N_CORES = 8
N, K, D = 131072, 100, 256
NL = N // N_CORES  # 16384 rows per core
P = 128            # partitions
TILES = NL // P    # 128 row-tiles per core
G = 16             # row-tiles per group (psum tile = G*128 f32 = 4 banks)
NGROUPS = TILES // G
NAUG = 3           # aug rows: z2_hi, z2_lo, ones

BF16 = mybir.dt.bfloat16
F32 = mybir.dt.float32
NP_BF16 = ml_dtypes.bfloat16

_CACHE = {}


def _build_program():
    nc = bacc.Bacc(
        "TRN2", target_bir_lowering=False, debug=False, num_devices=N_CORES
    )
    zt = nc.dram_tensor("zt", [D, NL], BF16, kind="ExternalInput").ap()
    zaug = nc.dram_tensor("zaug", [NAUG, NL], BF16, kind="ExternalInput").ap()
    cm = nc.dram_tensor("cm", [D, K], BF16, kind="ExternalInput").ap()
    caug = nc.dram_tensor("caug", [NAUG, K], BF16, kind="ExternalInput").ap()
    q = nc.dram_tensor("q", [NL, K], F32, kind="ExternalOutput").ap()

    GP = G * P  # columns of z per group

    with tile.TileContext(nc) as tc, ExitStack() as ctx:
        cpool = ctx.enter_context(tc.tile_pool(name="cpool", bufs=1))
        zpool = ctx.enter_context(tc.tile_pool(name="zpool", bufs=3))
        pspool = ctx.enter_context(tc.tile_pool(name="pspool", bufs=2, space="PSUM"))
        epool = ctx.enter_context(tc.tile_pool(name="epool", bufs=2))
        spool = ctx.enter_context(tc.tile_pool(name="spool", bufs=2))

        # Persistent moving operands (cluster side).
        cm0 = cpool.tile([P, K], BF16)
        nc.sync.dma_start(cm0[:], cm[0:P, :])
        cm1 = cpool.tile([P, K], BF16)
        nc.sync.dma_start(cm1[:], cm[P : 2 * P, :])
        cga = cpool.tile([NAUG, K], BF16)
        nc.sync.dma_start(cga[:], caug[:, :])
        # Persistent aug stationary rows (whole core's worth: 3 x 16384 bf16).
        zga = cpool.tile([NAUG, NL], BF16)
        nc.sync.dma_start(zga[:], zaug[:, :])

        for g in range(NGROUPS):
            c0, c1 = g * GP, (g + 1) * GP
            zA = zpool.tile([P, GP], BF16)
            nc.sync.dma_start(zA[:], zt[0:P, c0:c1])
            zB = zpool.tile([P, GP], BF16)
            nc.sync.dma_start(zB[:], zt[P : 2 * P, c0:c1])

            ps = pspool.tile([P, GP], F32)  # G*128 f32 = G/4 banks
            for t in range(G):
                sl_z = slice(t * P, (t + 1) * P)
                sl_o = slice(t * P, t * P + K)
                # 4 row-tiles fit in one 2KB psum bank / zero-region:
                # start on the bank's first matmul, stop on its last.
                nc.tensor.matmul(
                    ps[:, sl_o], zA[:, sl_z], cm0[:, :],
                    start=(t % 4 == 0), stop=False,
                )
                nc.tensor.matmul(
                    ps[:, sl_o], zB[:, sl_z], cm1[:, :],
                    start=False, stop=False,
                )
                nc.tensor.matmul(
                    ps[:, sl_o],
                    zga[:, c0 + t * P : c0 + (t + 1) * P],
                    cga[:, :],
                    start=False, stop=(t % 4 == 3),
                )

            # Epilogue, batched over the G row-tiles of this group.
            ps3 = ps.rearrange("p (t x) -> p t x", x=P)[:, :, 0:K]  # [128,G,100]
            qln = epool.tile([P, G * K], F32)
            qln3 = qln.rearrange("p (t k) -> p t k", k=K)
            nc.scalar.activation(qln3, ps3, mybir.ActivationFunctionType.Ln)
            qr = epool.tile([P, G * K], F32)
            nc.scalar.activation(
                qr[:], qln[:], mybir.ActivationFunctionType.Exp, scale=-1.0
            )
            qr3 = qr.rearrange("p (t k) -> p t k", k=K)
            rs = spool.tile([P, G], F32)
            nc.vector.tensor_reduce(
                rs[:], qr3, axis=mybir.AxisListType.X, op=mybir.AluOpType.add
            )
            rsi = spool.tile([P, G], F32)
            nc.vector.reciprocal(rsi[:], rs[:])
            # Broadcast rsi over the K dimension via a step-0 AP.
            rsi_b = bass.AP(
                rsi.tensor, rsi.offset, [list(rsi.ap[0]), list(rsi.ap[1]), [0, K]]
            )
            outt = epool.tile([P, G * K], F32)
            outt3 = outt.rearrange("p (t k) -> p t k", k=K)
            nc.vector.tensor_tensor(outt3, qr3, rsi_b, op=mybir.AluOpType.mult)

            # Store: q rows g*G*P .. (g+1)*G*P in natural [N,100] layout.
            q_g = q[g * GP : (g + 1) * GP, :].rearrange("(t p) k -> p t k", p=P)
            nc.sync.dma_start(q_g, outt3)

    nc.compile()
    return nc


def _prep_core_inputs(z_shard: np.ndarray, cm_np, caug_np) -> dict:
    # z_shard: [NL, D] f32
    zt = np.ascontiguousarray(z_shard.T.astype(NP_BF16))  # [D, NL]
    z2 = np.einsum("nd,nd->n", z_shard, z_shard, dtype=np.float32)  # [NL]
    z2_hi = z2.astype(NP_BF16)
    z2_lo = (z2 - z2_hi.astype(np.float32)).astype(NP_BF16)
    zaug = np.empty((NAUG, NL), dtype=NP_BF16)
    zaug[0] = z2_hi
    zaug[1] = z2_lo
    zaug[2] = np.ones((NL,), dtype=NP_BF16)
    return {"zt": zt, "zaug": zaug, "cm": cm_np, "caug": caug_np}


def kernel(z: np.ndarray, cluster_layer: np.ndarray) -> np.ndarray:
    assert z.shape == (N, D) and cluster_layer.shape == (K, D)
    z = np.asarray(z, dtype=np.float32)
    c = np.asarray(cluster_layer, dtype=np.float32)

    if "nc" not in _CACHE:
        _CACHE["nc"] = _build_program()
    nc = _CACHE["nc"]

    cm_np = np.ascontiguousarray((-2.0 * c.T).astype(NP_BF16))  # [D, K]
    c2 = np.einsum("kd,kd->k", c, c, dtype=np.float32)  # [K]
    caug_np = np.empty((NAUG, K), dtype=NP_BF16)
    caug_np[0] = 1.0
    caug_np[1] = 1.0
    caug_np[2] = (1.0 + c2).astype(NP_BF16)

    in_maps = [
        _prep_core_inputs(z[i * NL : (i + 1) * NL], cm_np, caug_np)
        for i in range(N_CORES)
    ]

    res = bass_utils.run_bass_kernel_spmd(
        nc, in_maps, core_ids=list(range(N_CORES))
    )
    out = np.concatenate([res.results[i]["q"] for i in range(N_CORES)], axis=0)
    return out.astype(np.float32)


# revision 12
# speedup vs baseline: 1.9905x; 1.9905x over previous
"""Trainium2 Bass kernel for nn_Clustering_Layer (retrieval_knn).

Computes q = row_normalize(1 / (1 + ||z - c_k||^2)) for z:[N,D], c:[K,D]
(Student-t / DEC clustering assignment, alpha=1).

Strategy (8 NeuronCores, data parallel over N):
  - Host: shard z along N; pre-transpose each shard to zT [D, N/8] (bf16) so
    the PE stationary operand loads directly from natural DRAM layout.
    Fold the distance expansion into one PSUM accumulation:
        denom = 1 + ||z||^2 + ||c_k||^2 - 2 z.c_k
    via 3 matmuls per 128-row tile:
        chunk A: zT[0:128]   x (-2 c^T)[0:128]
        chunk B: zT[128:256] x (-2 c^T)[128:256]
        aug    : [z2_hi; z2_lo; ones] x [ones; ones; 1+c2]
    z2 = ||z||^2 is computed on host in fp32 and shipped as two bf16 rows
    (hi + lo) so the dominant term keeps ~fp32 accuracy.
  - Rows are permuted host-side so that each SBUF partition ends up owning
    16 consecutive DRAM rows of q: the store then has 6.4KB-contiguous runs
    per partition instead of 400B (2x DMA throughput on TRN2 below 512B).
  - Device epilogue per group of 16 row-tiles (batched to amortize fixed
    per-instruction overheads): ACT Ln -> ACT Exp(-x) gives 1/denom (the
    banned-for-accuracy ACT Reciprocal is avoided; DVE's iterative-divide
    reciprocal is slower), DVE row-sum, row-sum reciprocal again via ACT
    Ln/Exp, DVE broadcast multiply. All ACT functions live in the single
    "natural_log_exp_and_others" table set; the activation-table map is
    restricted during compile so exactly one hoisted table load is emitted
    (instead of 12 alternating Ln/Exp set reloads).
  - Store q rows in natural [N,100] f32 layout (via the row permutation).
"""

import os
import sys
from contextlib import ExitStack, contextmanager

import numpy as np

for _p in ("/opt/trn_rl_repo", "/root/.axon_site/_ro/trn_rl_repo"):
    if os.path.isdir(_p) and _p not in sys.path:
        sys.path.insert(0, _p)

import ml_dtypes  # noqa: E402

import concourse.bass as bass  # noqa: E402
import concourse.tile as tile  # noqa: E402
from concourse import bacc, bass_utils, mybir  # noqa: E402

# Problem shape (hardcoded per spec).
N_CORES = 8
N, K, D = 131072, 100, 256
NL = N // N_CORES  # 16384 rows per core
P = 128            # partitions
TILES = NL // P    # 128 row-tiles per core
# Row-tiles per group: small head groups so the pipeline fills fast, large
# middle groups to amortize per-instruction overheads, small tail groups so
# the final serial MM->Ln->Exp->reduce->mul->store chain drains quickly.
GROUP_SIZES = [4, 8, 16, 16, 16, 16, 16, 16, 8, 8, 2, 2]
assert sum(GROUP_SIZES) == TILES
GMAX = max(GROUP_SIZES)
NAUG = 3              # aug rows: z2_hi, z2_lo, ones

BF16 = mybir.dt.bfloat16
F32 = mybir.dt.float32
NP_BF16 = ml_dtypes.bfloat16

# z and the cluster matrix ride in fp8-e3m4 (4 mantissa bits, range +-15.5;
# |z| <= ~6 for N(0,1) data). The whole denominator is scaled by SCALE so the
# small cluster values leave the e3m4 subnormal range; q is invariant to a
# uniform scale of the denominators (it cancels in the row normalization).
Z_DT = mybir.dt.float8e3
NP_Z = ml_dtypes.float8_e3m4
SCALE = 16.0

COMBINED_ACT_SET = "natural_log_exp_and_others"

_CACHE = {}


@contextmanager
def _single_act_table():
    """Restrict Ln/Exp to the combined table set during bacc's act-table-load
    insertion so one hoisted InstLoadActFuncSet is emitted instead of
    alternating per-function set reloads."""
    import concourse.bacc as bacc_mod

    orig = bacc_mod.get_activation_tables

    def patched(module_arch):
        tabs = orig(module_arch)
        ln = mybir.ActivationFunctionType.Ln
        ex = mybir.ActivationFunctionType.Exp
        out = {}
        for name, funcs in tabs.items():
            if name == COMBINED_ACT_SET:
                out[name] = funcs
            else:
                out[name] = {f for f in funcs if f not in (ln, ex)}
        return out

    bacc_mod.get_activation_tables = patched
    try:
        yield
    finally:
        bacc_mod.get_activation_tables = orig


def _build_program():
    nc = bacc.Bacc(
        "TRN2", target_bir_lowering=False, debug=False, num_devices=N_CORES
    )
    zt = nc.dram_tensor("zt", [D, NL], Z_DT, kind="ExternalInput").ap()
    zaug = nc.dram_tensor("zaug", [NAUG, NL], BF16, kind="ExternalInput").ap()
    cm = nc.dram_tensor("cm", [D, K], BF16, kind="ExternalInput").ap()
    caug = nc.dram_tensor("caug", [NAUG, K], BF16, kind="ExternalInput").ap()
    q = nc.dram_tensor("q", [NL, K], F32, kind="ExternalOutput").ap()

    with tile.TileContext(nc) as tc, ExitStack() as ctx:
        cpool = ctx.enter_context(tc.tile_pool(name="cpool", bufs=1))
        zpool = ctx.enter_context(tc.tile_pool(name="zpool", bufs=4))
        pspool = ctx.enter_context(tc.tile_pool(name="pspool", bufs=2, space="PSUM"))
        epool = ctx.enter_context(tc.tile_pool(name="epool", bufs=5))
        spool = ctx.enter_context(tc.tile_pool(name="spool", bufs=5))

        # Persistent moving operands (cluster side), one merged DMA issued
        # from the otherwise-idle ACT sequencer so the head-of-kernel
        # transfers pipeline through the HWDGE in parallel with the z loads
        # issued from SP.
        cmall = cpool.tile([P, 2, K], BF16)
        nc.scalar.dma_start(
            cmall[:], cm.rearrange("(h p) k -> p h k", p=P)
        )
        cga = cpool.tile([NAUG, K], BF16)
        nc.scalar.dma_start(cga[:], caug[:, :])
        # Persistent aug stationary rows (whole core's worth: 3 x 16384
        # bf16). Issued via GPSIMD's SWDGE path (only 3 descriptors): it
        # bypasses the HWDGE queue entirely, so the first group's aug
        # matmuls aren't gated behind the z loads.
        zga = cpool.tile([NAUG, NL], BF16)
        nc.gpsimd.dma_start(zga[:], zaug[:, :])
        cm0 = cmall[:, 0, :]
        cm1 = cmall[:, 1, :]

        goff = 0
        for g, GS in enumerate(GROUP_SIZES):
            GPg = GS * P
            c0, c1 = goff, goff + GPg
            zAB = zpool.tile([P, 2, GPg], Z_DT, tag="zAB")
            nc.sync.dma_start(
                zAB[:], zt[:, c0:c1].rearrange("(h p) j -> p h j", p=P)
            )
            zA = zAB[:, 0, :]
            zB = zAB[:, 1, :]

            ps = pspool.tile([P, GPg], F32, tag="ps")
            for t in range(GS):
                sl_z = slice(t * P, (t + 1) * P)
                sl_o = slice(t * P, t * P + K)
                # 4 row-tiles fit in one 2KB psum bank / zero-region:
                # start on the bank's first matmul, stop on its last.
                nc.tensor.matmul(
                    ps[:, sl_o], zA[:, sl_z], cm0,
                    start=(t % 4 == 0), stop=False,
                )
                nc.tensor.matmul(
                    ps[:, sl_o], zB[:, sl_z], cm1,
                    start=False, stop=False,
                )
                nc.tensor.matmul(
                    ps[:, sl_o],
                    zga[:, c0 + t * P : c0 + (t + 1) * P],
                    cga[:, :],
                    start=False, stop=(t % 4 == 3 or t == GS - 1),
                )

            # Epilogue, batched over the GS row-tiles of this group.
            ps3 = ps.rearrange("p (t x) -> p t x", x=P)[:, :, 0:K]  # [128,GS,100]
            qln = epool.tile([P, GS * K], F32, tag="qln")
            qln3 = qln.rearrange("p (t k) -> p t k", k=K)
            nc.scalar.activation(qln3, ps3, mybir.ActivationFunctionType.Ln)
            qr = epool.tile([P, GS * K], F32, tag="qr")
            nc.scalar.activation(
                qr[:], qln[:], mybir.ActivationFunctionType.Exp, scale=-1.0
            )
            qr3 = qr.rearrange("p (t k) -> p t k", k=K)
            rs = spool.tile([P, GS], F32, tag="rs")
            nc.vector.tensor_reduce(
                rs[:], qr3, axis=mybir.AxisListType.X, op=mybir.AluOpType.add
            )
            # 1/rowsum on DVE (tiny op; keeps the reduce->recip->mult chain
            # on one engine and the big Ln/Exp passes on ACT).
            rsi = spool.tile([P, GS], F32, tag="rsi")
            nc.vector.reciprocal(rsi[:], rs[:])
            # Broadcast rsi over the K dimension via a step-0 AP.
            rsi_b = bass.AP(
                rsi.tensor, rsi.offset, [list(rsi.ap[0]), list(rsi.ap[1]), [0, K]]
            )
            outt = epool.tile([P, GS * K], F32, tag="qln")
            outt3 = outt.rearrange("p (t k) -> p t k", k=K)
            nc.vector.tensor_tensor(outt3, qr3, rsi_b, op=mybir.AluOpType.mult)

            # Store. Host-side row permutation arranged row (goff + p*GS + t)
            # into out[p, t]: per-partition runs are GS*K*4 contiguous bytes
            # in DRAM (>= 800B, above the 512B full-rate threshold). 2D AP
            # (rows merge with columns) keeps the descriptor-gen cost low.
            q_g = q[goff : goff + GPg, :].rearrange("(p t) k -> p (t k)", t=GS)
            nc.sync.dma_start(q_g, outt[:])
            goff += GPg

    with _single_act_table():
        nc.compile()
    return nc


def _permute_rows(z_shard: np.ndarray) -> np.ndarray:
    """Reorder rows so device row-tile t of group g holds original rows
    {goff + p*GS + t : p in 0..127}; i.e. feed row (goff + t*P + p) :=
    original row (goff + p*GS + t)."""
    out = np.empty_like(z_shard)
    off = 0
    for GS in GROUP_SIZES:
        n = GS * P
        blk = z_shard[off : off + n].reshape(P, GS, -1)   # [p, t, D]
        out[off : off + n] = blk.transpose(1, 0, 2).reshape(n, -1)
        off += n
    return out


def _prep_core_inputs(z_shard: np.ndarray, cm_np, caug_np) -> dict:
    # z_shard: [NL, D] f32
    zp = _permute_rows(z_shard)
    zt = np.ascontiguousarray(zp.T.astype(NP_Z))  # [D, NL]
    z2 = np.einsum("nd,nd->n", zp, zp, dtype=np.float32)  # [NL]
    z2_hi = z2.astype(NP_BF16)
    z2_lo = (z2 - z2_hi.astype(np.float32)).astype(NP_BF16)
    zaug = np.empty((NAUG, NL), dtype=NP_BF16)
    zaug[0] = z2_hi
    zaug[1] = z2_lo
    zaug[2] = np.ones((NL,), dtype=NP_BF16)
    return {"zt": zt, "zaug": zaug, "cm": cm_np, "caug": caug_np}


def kernel(z: np.ndarray, cluster_layer: np.ndarray) -> np.ndarray:
    assert z.shape == (N, D) and cluster_layer.shape == (K, D)
    z = np.asarray(z, dtype=np.float32)
    c = np.asarray(cluster_layer, dtype=np.float32)

    if "nc" not in _CACHE:
        _CACHE["nc"] = _build_program()
    nc = _CACHE["nc"]

    cm_np = np.ascontiguousarray((-2.0 * SCALE * c.T).astype(NP_BF16))  # [D, K]
    c2 = np.einsum("kd,kd->k", c, c, dtype=np.float32)  # [K]
    caug_np = np.empty((NAUG, K), dtype=NP_BF16)
    caug_np[0] = SCALE
    caug_np[1] = SCALE
    caug_np[2] = (SCALE * (1.0 + c2)).astype(NP_BF16)

    in_maps = [
        _prep_core_inputs(z[i * NL : (i + 1) * NL], cm_np, caug_np)
        for i in range(N_CORES)
    ]

    res = bass_utils.run_bass_kernel_spmd(
        nc, in_maps, core_ids=list(range(N_CORES))
    )
    out = np.concatenate([res.results[i]["q"] for i in range(N_CORES)], axis=0)
    return out.astype(np.float32)


# revision 19
# speedup vs baseline: 1.9983x; 1.0039x over previous
"""Trainium2 Bass kernel for nn_Clustering_Layer (retrieval_knn).

Computes q = row_normalize(1 / (1 + ||z - c_k||^2)) for z:[N,D], c:[K,D]
(Student-t / DEC clustering assignment, alpha=1).

Strategy (8 NeuronCores, data parallel over N):
  - Host: shard z along N; pre-transpose each shard to zT [D, N/8] (fp8
    e3m4, see Z_DT below) so the PE stationary operand loads directly from
    natural DRAM layout. Fold the whole distance expansion into one PSUM
    accumulation:
        denom = SCALE * (1 + ||z||^2 + ||c_k||^2 - 2 z.c_k)
    via 3 matmuls per 128-row tile:
        chunk A: zT[0:128]   x (-2*SCALE c^T)[0:128]
        chunk B: zT[128:256] x (-2*SCALE c^T)[128:256]
        aug    : [z2_hi; z2_lo; ones] x [S; S; S*(1+c2)]
    z2 = ||z||^2 is computed on host in fp32 and shipped as two bf16 rows
    (hi + lo) so the dominant term keeps ~fp32 accuracy.
  - Rows are permuted host-side so that each SBUF partition ends up owning
    the GS consecutive DRAM rows of q its group writes: stores then have
    GS*400B-contiguous runs per partition instead of 400B (DMA below 512B
    contiguity runs at half rate on TRN2).
  - Device epilogue per group of row-tiles (batched to amortize fixed
    per-instruction overheads): ACT Ln -> ACT Exp(-x) gives 1/denom (the
    banned-for-accuracy ACT Reciprocal is avoided), DVE row-sum + tiny
    reciprocal + broadcast multiply. Ln and Exp both live in the
    "natural_log_exp_and_others" table set; the activation-table map is
    restricted during compile so the table is loaded once instead of
    thrashing between per-function sets every group.
  - Store q rows in natural [N,100] f32 layout (via the row permutation).
"""

import os
import sys
from contextlib import ExitStack, contextmanager

import numpy as np

for _p in ("/opt/trn_rl_repo", "/root/.axon_site/_ro/trn_rl_repo"):
    if os.path.isdir(_p) and _p not in sys.path:
        sys.path.insert(0, _p)

import ml_dtypes  # noqa: E402

import concourse.bass as bass  # noqa: E402
import concourse.tile as tile  # noqa: E402
from concourse import bacc, bass_utils, mybir  # noqa: E402

# Problem shape (hardcoded per spec).
N_CORES = 8
N, K, D = 131072, 100, 256
NL = N // N_CORES  # 16384 rows per core
P = 128            # partitions
TILES = NL // P    # 128 row-tiles per core
# Row-tiles per group: small head groups so the pipeline fills fast, large
# middle groups to amortize per-instruction overheads, small tail groups so
# the final serial MM->Ln->Exp->reduce->mul->store chain drains quickly.
GROUP_SIZES = [4, 16, 16, 16, 16, 16, 16, 16, 8, 2, 2]
assert sum(GROUP_SIZES) == TILES
GMAX = max(GROUP_SIZES)
NAUG = 3              # aug rows: z2_hi, z2_lo, ones

BF16 = mybir.dt.bfloat16
F32 = mybir.dt.float32
NP_BF16 = ml_dtypes.bfloat16

# z and the cluster matrix ride in fp8-e3m4 (4 mantissa bits, range +-15.5;
# |z| <= ~6 for N(0,1) data). The whole denominator is scaled by SCALE so the
# small cluster values leave the e3m4 subnormal range; q is invariant to a
# uniform scale of the denominators (it cancels in the row normalization).
Z_DT = mybir.dt.float8e3
NP_Z = ml_dtypes.float8_e3m4
SCALE = 16.0

COMBINED_ACT_SET = "natural_log_exp_and_others"

_CACHE = {}


@contextmanager
def _single_act_table():
    """Restrict Ln/Exp to the combined table set during bacc's act-table-load
    insertion so one hoisted InstLoadActFuncSet is emitted instead of
    alternating per-function set reloads."""
    import concourse.bacc as bacc_mod

    orig = bacc_mod.get_activation_tables

    def patched(module_arch):
        tabs = orig(module_arch)
        ln = mybir.ActivationFunctionType.Ln
        ex = mybir.ActivationFunctionType.Exp
        combined = tabs.get(COMBINED_ACT_SET)
        if combined is None or ln not in combined or ex not in combined:
            return tabs  # unknown act_info layout: leave untouched
        out = {}
        for name, funcs in tabs.items():
            if name == COMBINED_ACT_SET:
                out[name] = funcs
            else:
                out[name] = {f for f in funcs if f not in (ln, ex)}
        return out

    bacc_mod.get_activation_tables = patched
    try:
        yield
    finally:
        bacc_mod.get_activation_tables = orig


def _build_program():
    nc = bacc.Bacc(
        "TRN2", target_bir_lowering=False, debug=False, num_devices=N_CORES
    )
    zt = nc.dram_tensor("zt", [D, NL], Z_DT, kind="ExternalInput").ap()
    zaug = nc.dram_tensor("zaug", [NAUG, NL], BF16, kind="ExternalInput").ap()
    cm = nc.dram_tensor("cm", [D, K], BF16, kind="ExternalInput").ap()
    caug = nc.dram_tensor("caug", [NAUG, K], BF16, kind="ExternalInput").ap()
    q = nc.dram_tensor("q", [NL, K], F32, kind="ExternalOutput").ap()

    with tile.TileContext(nc) as tc, ExitStack() as ctx:
        cpool = ctx.enter_context(tc.tile_pool(name="cpool", bufs=1))
        zpool = ctx.enter_context(tc.tile_pool(name="zpool", bufs=4))
        pspool = ctx.enter_context(tc.tile_pool(name="pspool", bufs=2, space="PSUM"))
        epool = ctx.enter_context(tc.tile_pool(name="epool", bufs=5))
        spool = ctx.enter_context(tc.tile_pool(name="spool", bufs=5))

        # Persistent moving operands (cluster side), one merged DMA issued
        # from the otherwise-idle ACT sequencer so the head-of-kernel
        # transfers pipeline through the HWDGE in parallel with the z loads
        # issued from SP.
        cmall = cpool.tile([P, 2, K], BF16)
        nc.scalar.dma_start(
            cmall[:], cm.rearrange("(h p) k -> p h k", p=P)
        )
        cga = cpool.tile([NAUG, K], BF16)
        nc.scalar.dma_start(cga[:], caug[:, :])
        # Persistent aug stationary rows (whole core's worth: 3 x 16384
        # bf16). Issued via GPSIMD's SWDGE path (only 3 descriptors): it
        # bypasses the HWDGE queue entirely, so the first group's aug
        # matmuls aren't gated behind the z loads.
        zga = cpool.tile([NAUG, NL], BF16)
        nc.gpsimd.dma_start(zga[:], zaug[:, :])
        cm0 = cmall[:, 0, :]
        cm1 = cmall[:, 1, :]

        goff = 0
        for g, GS in enumerate(GROUP_SIZES):
            GPg = GS * P
            c0, c1 = goff, goff + GPg
            zAB = zpool.tile([P, 2, GPg], Z_DT, tag="zAB")
            nc.sync.dma_start(
                zAB[:], zt[:, c0:c1].rearrange("(h p) j -> p h j", p=P)
            )
            zA = zAB[:, 0, :]
            zB = zAB[:, 1, :]

            ps = pspool.tile([P, GPg], F32, tag="ps")
            for t in range(GS):
                sl_z = slice(t * P, (t + 1) * P)
                sl_o = slice(t * P, t * P + K)
                # 4 row-tiles fit in one 2KB psum bank / zero-region:
                # start on the bank's first matmul, stop on its last.
                nc.tensor.matmul(
                    ps[:, sl_o], zA[:, sl_z], cm0,
                    start=(t % 4 == 0), stop=False,
                )
                nc.tensor.matmul(
                    ps[:, sl_o], zB[:, sl_z], cm1,
                    start=False, stop=False,
                )
                nc.tensor.matmul(
                    ps[:, sl_o],
                    zga[:, c0 + t * P : c0 + (t + 1) * P],
                    cga[:, :],
                    start=False, stop=(t % 4 == 3 or t == GS - 1),
                )

            # Epilogue, batched over the GS row-tiles of this group.
            ps3 = ps.rearrange("p (t x) -> p t x", x=P)[:, :, 0:K]  # [128,GS,100]
            qln = epool.tile([P, GS * K], F32, tag="qln")
            qln3 = qln.rearrange("p (t k) -> p t k", k=K)
            nc.scalar.activation(qln3, ps3, mybir.ActivationFunctionType.Ln)
            qr = epool.tile([P, GS * K], F32, tag="qr")
            nc.scalar.activation(
                qr[:], qln[:], mybir.ActivationFunctionType.Exp, scale=-1.0
            )
            qr3 = qr.rearrange("p (t k) -> p t k", k=K)
            rs = spool.tile([P, GS], F32, tag="rs")
            nc.vector.tensor_reduce(
                rs[:], qr3, axis=mybir.AxisListType.X, op=mybir.AluOpType.add
            )
            # 1/rowsum on DVE (tiny op; keeps the reduce->recip->mult chain
            # on one engine and the big Ln/Exp passes on ACT).
            rsi = spool.tile([P, GS], F32, tag="rsi")
            nc.vector.reciprocal(rsi[:], rs[:])
            # Broadcast rsi over the K dimension via a step-0 AP.
            rsi_b = bass.AP(
                rsi.tensor, rsi.offset, [list(rsi.ap[0]), list(rsi.ap[1]), [0, K]]
            )
            outt = epool.tile([P, GS * K], F32, tag="qln")
            outt3 = outt.rearrange("p (t k) -> p t k", k=K)
            nc.vector.tensor_tensor(outt3, qr3, rsi_b, op=mybir.AluOpType.mult)

            # Store. Host-side row permutation arranged row (goff + p*GS + t)
            # into out[p, t]: per-partition runs are GS*K*4 contiguous bytes
            # in DRAM (>= 800B, above the 512B full-rate threshold). 2D AP
            # (rows merge with columns) keeps the descriptor-gen cost low.
            q_g = q[goff : goff + GPg, :].rearrange("(p t) k -> p (t k)", t=GS)
            nc.sync.dma_start(q_g, outt[:])
            goff += GPg

    with _single_act_table():
        nc.compile()
    return nc


def _permute_rows(z_shard: np.ndarray) -> np.ndarray:
    """Reorder rows so device row-tile t of group g holds original rows
    {goff + p*GS + t : p in 0..127}; i.e. feed row (goff + t*P + p) :=
    original row (goff + p*GS + t)."""
    out = np.empty_like(z_shard)
    off = 0
    for GS in GROUP_SIZES:
        n = GS * P
        blk = z_shard[off : off + n].reshape(P, GS, -1)   # [p, t, D]
        out[off : off + n] = blk.transpose(1, 0, 2).reshape(n, -1)
        off += n
    return out


def _prep_core_inputs(z_shard: np.ndarray, cm_np, caug_np) -> dict:
    # z_shard: [NL, D] f32
    zp = _permute_rows(z_shard)
    zt = np.ascontiguousarray(zp.T.astype(NP_Z))  # [D, NL]
    z2 = np.einsum("nd,nd->n", zp, zp, dtype=np.float32)  # [NL]
    z2_hi = z2.astype(NP_BF16)
    z2_lo = (z2 - z2_hi.astype(np.float32)).astype(NP_BF16)
    zaug = np.empty((NAUG, NL), dtype=NP_BF16)
    zaug[0] = z2_hi
    zaug[1] = z2_lo
    zaug[2] = np.ones((NL,), dtype=NP_BF16)
    return {"zt": zt, "zaug": zaug, "cm": cm_np, "caug": caug_np}


def kernel(z: np.ndarray, cluster_layer: np.ndarray) -> np.ndarray:
    assert z.shape == (N, D) and cluster_layer.shape == (K, D)
    z = np.asarray(z, dtype=np.float32)
    c = np.asarray(cluster_layer, dtype=np.float32)

    if "nc" not in _CACHE:
        _CACHE["nc"] = _build_program()
    nc = _CACHE["nc"]

    cm_np = np.ascontiguousarray((-2.0 * SCALE * c.T).astype(NP_BF16))  # [D, K]
    c2 = np.einsum("kd,kd->k", c, c, dtype=np.float32)  # [K]
    caug_np = np.empty((NAUG, K), dtype=NP_BF16)
    caug_np[0] = SCALE
    caug_np[1] = SCALE
    caug_np[2] = (SCALE * (1.0 + c2)).astype(NP_BF16)

    in_maps = [
        _prep_core_inputs(z[i * NL : (i + 1) * NL], cm_np, caug_np)
        for i in range(N_CORES)
    ]

    res = bass_utils.run_bass_kernel_spmd(
        nc, in_maps, core_ids=list(range(N_CORES))
    )
    out = np.concatenate([res.results[i]["q"] for i in range(N_CORES)], axis=0)
    return out.astype(np.float32)
